# revision 5
# baseline (speedup 1.0000x reference)
"""Trainium2 Bass kernel for nn_CausalGraphLearner.

Computes scores[i,j] = mean_b sigmoid(W2 . gelu(ctx[b] + cause[i] + effect[j] + b1) + b2)
with B=64, V=64, DIM=512, H=1024.

Sharding: data-parallel over B across 8 NeuronCores (8 batch rows per core);
embed / W1 / b1 / W2 / b2 are replicated. Each core emits the partial sum
sum_{b in shard} sigmoid(logits[b]) / 64 as a [1, 4096] tensor; the host sums
the 8 partials and reshapes to [64, 64].

Per-core plan (engines):
  - PE: phase-1 transposes + cause/effect/ctx projection matmuls (f32),
        main-loop logits contraction vs W2 (bf16, N=512 slices into PSUM),
        tail ones-matmul to sum the 8 local batch rows.
  - DVE: builds the pairwise table P[c][h, i, j] = cause[h,i] + effect[h,j]
        (broadcast tensor_tensor straight from PSUM, bf16 out), PSUM->SBUF
        logits copies, small glue.
  - ACT (the roofline engine, ~230us/core): 64 x gelu(P[c] + CB[c,b]) over
        [128 x 4096], plus one final sigmoid over [8 x 4096].
"""

import sys

if "/opt/trn_rl_repo" not in sys.path:
    sys.path.insert(0, "/opt/trn_rl_repo")

import numpy as np

B, V, DIM = 64, 64, 512
H = 2 * DIM
N_CORES = 8
BS = B // N_CORES          # 8 batch rows per core
KC = DIM // 128            # 4 contraction chunks
HC = H // 128              # 8 hidden chunks
IJ = V * V                 # 4096

_CACHE = {}


def _build_nc():
    import concourse.bacc as bacc
    import concourse.bass as bass
    import concourse.mybir as mybir
    import concourse.tile as tile
    from concourse.masks import make_identity

    f32 = mybir.dt.float32
    bf16 = mybir.dt.bfloat16
    Gelu = mybir.ActivationFunctionType.Gelu
    Sigmoid = mybir.ActivationFunctionType.Sigmoid

    nc = bacc.Bacc("TRN2", target_bir_lowering=False, debug=False)

    st_d = nc.dram_tensor("state_s", [BS, DIM], f32, kind="ExternalInput")
    ac_d = nc.dram_tensor("action_s", [BS, DIM], f32, kind="ExternalInput")
    em_d = nc.dram_tensor("embed", [V, DIM], f32, kind="ExternalInput")
    w1_d = nc.dram_tensor("W1", [3 * DIM, H], f32, kind="ExternalInput")
    b1_d = nc.dram_tensor("b1", [H], f32, kind="ExternalInput")
    w2_d = nc.dram_tensor("W2", [H, 1], f32, kind="ExternalInput")
    b2_d = nc.dram_tensor("b2", [1], f32, kind="ExternalInput")
    out_d = nc.dram_tensor("out", [1, IJ], f32, kind="ExternalOutput")

    with tile.TileContext(nc) as tc:
        with (
            tc.tile_pool(name="singles", bufs=1) as singles,
            tc.tile_pool(name="wpool", bufs=2) as wpool,
            tc.tile_pool(name="actp", bufs=3) as actp,
            tc.tile_pool(name="scrp", bufs=2) as scrp,
        ):
            ident = singles.tile([128, 128], f32)
            make_identity(nc, ident[:, :])

            e_raw = singles.tile([V, DIM], f32)
            nc.sync.dma_start(out=e_raw[:, :], in_=em_d[:, :])
            st_raw = singles.tile([BS, DIM], f32)
            nc.sync.dma_start(out=st_raw[:, :], in_=st_d[:, :])
            ac_raw = singles.tile([BS, DIM], f32)
            nc.sync.dma_start(out=ac_raw[:, :], in_=ac_d[:, :])
            # b1 / W2 loaded chunk-column-major: col c = elements [128c, 128c+128)
            b1_sb = singles.tile([128, HC], f32)
            nc.sync.dma_start(
                out=b1_sb[:, :], in_=b1_d.rearrange("(c p) -> p c", p=128)
            )
            w2_sb = singles.tile([128, HC], f32)
            nc.sync.dma_start(
                out=w2_sb[:, :], in_=w2_d.rearrange("(c p) o -> p (c o)", p=128)
            )
            b2_sb = singles.tile([BS, 1], f32)
            nc.sync.dma_start(out=b2_sb[:, :], in_=b2_d[:].to_broadcast((BS, 1)))

            w2_bf = singles.tile([128, HC], bf16)
            nc.vector.tensor_copy(out=w2_bf[:, :], in_=w2_sb[:, :])

            sa = singles.tile([BS, DIM], f32)
            nc.vector.tensor_add(out=sa[:, :], in0=st_raw[:, :], in1=ac_raw[:, :])

            embT = singles.tile([128, KC, V], f32)     # embed^T, k-chunked
            saT = singles.tile([128, KC, BS], f32)     # (state+action)^T, k-chunked
            P = singles.tile([128, HC, V, V], bf16)    # cause (+) effect pairwise table
            CB = singles.tile([128, HC, BS], f32)      # ctx_hT + b1, per-(chunk, b) bias
            L = singles.tile([BS, IJ], f32)            # logits, batch-major
            S = singles.tile([BS, IJ], bf16)           # sigmoid(logits)
            ones = singles.tile([BS, 1], bf16)
            nc.vector.memset(ones[:, :], 1.0)
            out_sb = singles.tile([1, IJ], f32)

            with tc.tile_pool(name="psum1", bufs=2, space=bass.MemorySpace.PSUM) as psum1:
                for k in range(KC):
                    pt = psum1.tile([128, V], f32, tag="pt")
                    nc.tensor.transpose(
                        out=pt[:, :],
                        in_=e_raw[:, k * 128:(k + 1) * 128],
                        identity=ident[:V, :V],
                    )
                    nc.vector.tensor_copy(out=embT[:, k, :], in_=pt[:, :])
                for k in range(KC):
                    pt2 = psum1.tile([128, BS], f32, tag="pt")
                    nc.tensor.transpose(
                        out=pt2[:, :],
                        in_=sa[:, k * 128:(k + 1) * 128],
                        identity=ident[:BS, :BS],
                    )
                    nc.vector.tensor_copy(out=saT[:, k, :], in_=pt2[:, :])

                for c in range(HC):
                    pca = psum1.tile([128, V], f32, tag="pca")   # cause_hT chunk
                    pce = psum1.tile([128, V], f32, tag="pce")   # effect_hT chunk
                    pcx = psum1.tile([128, BS], f32, tag="pcx")  # ctx_hT chunk
                    for k in range(KC):
                        wt_c = wpool.tile([128, 128], f32, tag="wc")
                        nc.sync.dma_start(
                            out=wt_c[:, :],
                            in_=w1_d[k * 128:(k + 1) * 128, c * 128:(c + 1) * 128],
                        )
                        nc.tensor.matmul(
                            pca[:, :], lhsT=wt_c[:, :], rhs=embT[:, k, :],
                            start=(k == 0), stop=(k == KC - 1),
                        )
                        wt_e = wpool.tile([128, 128], f32, tag="we")
                        nc.sync.dma_start(
                            out=wt_e[:, :],
                            in_=w1_d[DIM + k * 128:DIM + (k + 1) * 128,
                                     c * 128:(c + 1) * 128],
                        )
                        nc.tensor.matmul(
                            pce[:, :], lhsT=wt_e[:, :], rhs=embT[:, k, :],
                            start=(k == 0), stop=(k == KC - 1),
                        )
                        wt_x = wpool.tile([128, 128], f32, tag="wx")
                        nc.sync.dma_start(
                            out=wt_x[:, :],
                            in_=w1_d[2 * DIM + k * 128:2 * DIM + (k + 1) * 128,
                                     c * 128:(c + 1) * 128],
                        )
                        nc.tensor.matmul(
                            pcx[:, :], lhsT=wt_x[:, :], rhs=saT[:, k, :],
                            start=(k == 0), stop=(k == KC - 1),
                        )
                    nc.vector.tensor_scalar_add(
                        out=CB[:, c, :], in0=pcx[:, :], scalar1=b1_sb[:, c:c + 1]
                    )
                    # DVE may read only ONE non-scalar operand from PSUM, so
                    # stage cause in SBUF before the broadcast pairwise add.
                    cau = wpool.tile([128, V], f32, tag="cau", name=f"cau{c}")
                    nc.vector.tensor_copy(out=cau[:, :], in_=pca[:, :])
                    # P[c][p, i, j] = effect[p, j] + cause[p, i]
                    nc.vector.tensor_add(
                        out=P[:, c, :, :],
                        in0=pce[:, None, :].to_broadcast((128, V, V)),
                        in1=cau[:, :, None].to_broadcast((128, V, V)),
                    )

            with tc.tile_pool(name="psum2", bufs=8, space=bass.MemorySpace.PSUM) as psum2:
                for b in range(BS):
                    pls = [
                        psum2.tile([1, 512], f32, tag="pl", name=f"pl{b}_{s}")
                        for s in range(8)
                    ]
                    for c in range(HC):
                        act = actp.tile([128, V, V], bf16, tag="act")
                        nc.scalar.activation(
                            out=act[:, :, :],
                            in_=P[:, c, :, :],
                            func=Gelu,
                            bias=CB[:, c, b:b + 1],
                            scale=1.0,
                        )
                        for s in range(8):
                            nc.tensor.matmul(
                                pls[s][:, :],
                                lhsT=w2_bf[:, c:c + 1],
                                rhs=act[:, 8 * s:8 * (s + 1), :],
                                start=(c == 0), stop=(c == HC - 1),
                            )
                    scr = scrp.tile([1, IJ], f32, tag="scr")
                    for s in range(8):
                        nc.vector.tensor_copy(
                            out=scr[:, 512 * s:512 * (s + 1)], in_=pls[s][:, :]
                        )
                    # engines can only address SBUF partitions {0,32,64,96}; DMA
                    # is exempt, so place row b of L with an SBUF->SBUF DMA
                    nc.sync.dma_start(out=L[b:b + 1, :], in_=scr[:, :])

                nc.scalar.activation(
                    out=S[:, :], in_=L[:, :], func=Sigmoid, bias=b2_sb[:, :], scale=1.0
                )
                for s in range(8):
                    po = psum2.tile([1, 512], f32, tag="pl")
                    nc.tensor.matmul(
                        po[:, :], lhsT=ones[:, :], rhs=S[:, 512 * s:512 * (s + 1)],
                        start=True, stop=True,
                    )
                    nc.vector.tensor_scalar_mul(
                        out=out_sb[:, 512 * s:512 * (s + 1)], in0=po[:, :],
                        scalar1=1.0 / B,
                    )
                nc.sync.dma_start(out=out_d[:, :], in_=out_sb[:, :])

    nc.compile()
    return nc


def _get_nc():
    if "nc" not in _CACHE:
        _CACHE["nc"] = _build_nc()
    return _CACHE["nc"]


def _make_in_maps(inputs):
    state = np.ascontiguousarray(np.asarray(inputs["state"], dtype=np.float32))
    action = np.ascontiguousarray(np.asarray(inputs["action"], dtype=np.float32))
    embed = np.ascontiguousarray(np.asarray(inputs["embed"], dtype=np.float32))
    W1 = np.ascontiguousarray(np.asarray(inputs["W1"], dtype=np.float32))
    b1 = np.ascontiguousarray(np.asarray(inputs["b1"], dtype=np.float32))
    W2 = np.ascontiguousarray(np.asarray(inputs["W2"], dtype=np.float32))
    b2 = np.ascontiguousarray(np.asarray(inputs["b2"], dtype=np.float32))
    in_maps = []
    for c in range(N_CORES):
        in_maps.append({
            "state_s": np.ascontiguousarray(state[c * BS:(c + 1) * BS]),
            "action_s": np.ascontiguousarray(action[c * BS:(c + 1) * BS]),
            "embed": embed,
            "W1": W1,
            "b1": b1,
            "W2": W2,
            "b2": b2,
        })
    return in_maps


def _ensure_ntff_hook():
    """This image's antenv lacks axon_hooks; synthesize it from the boot shim
    so run_bass_kernel_spmd(trace=True) can capture NTFF profiles."""
    import types

    try:
        from antenv.axon_hooks import get_axon_ntff_profile_hook  # noqa: F401
        return True
    except ImportError:
        pass
    try:
        if "/root/.axon_site" not in sys.path:
            sys.path.insert(0, "/root/.axon_site")
        from trn_agent_boot.trn_boot import _ntff_profile_via_ctypes

        hook = _ntff_profile_via_ctypes("/opt/axon/libaxon_pjrt.so")
    except Exception:
        hook = None
    if hook is None:
        return False
    import antenv

    mod = types.ModuleType("antenv.axon_hooks")
    mod._hook = hook
    mod.get_axon_ntff_profile_hook = lambda: mod._hook

    def set_axon_ntff_profile_hook(h):
        mod._hook = h

    mod.set_axon_ntff_profile_hook = set_axon_ntff_profile_hook
    sys.modules["antenv.axon_hooks"] = mod
    antenv.axon_hooks = mod
    return True


def run_sharded(inputs, trace=False, **kwargs):
    """Run the SPMD kernel on 8 cores; returns (scores [V,V] f32, BassKernelResults)."""
    from concourse.bass_utils import run_bass_kernel_spmd

    if trace:
        _ensure_ntff_hook()
    nc = _get_nc()
    in_maps = _make_in_maps(inputs)
    res = run_bass_kernel_spmd(
        nc, in_maps, core_ids=list(range(N_CORES)), trace=trace, **kwargs
    )
    parts = np.stack([res.results[c]["out"].reshape(V, V) for c in range(N_CORES)])
    scores = parts.astype(np.float64).sum(axis=0).astype(np.float32)
    return scores, res


def kernel(**inputs) -> np.ndarray:
    scores, _ = run_sharded(inputs, trace=False)
    return scores


if __name__ == "__main__":
    rng = np.random.default_rng(0)
    demo = {
        "state": rng.standard_normal((B, DIM), dtype=np.float32),
        "action": rng.standard_normal((B, DIM), dtype=np.float32),
        "embed": rng.standard_normal((V, DIM), dtype=np.float32),
        "W1": (rng.standard_normal((3 * DIM, H)) * 0.05).astype(np.float32),
        "b1": (rng.standard_normal((H,)) * 0.05).astype(np.float32),
        "W2": (rng.standard_normal((H, 1)) * 0.05).astype(np.float32),
        "b2": (rng.standard_normal((1,)) * 0.05).astype(np.float32),
    }
    out = kernel(**demo)
    print(out.shape, out.dtype, out[:2, :4])


# revision 9
# speedup vs baseline: 1.0608x; 1.0608x over previous
"""Trainium2 Bass kernel for nn_CausalGraphLearner.

Computes scores[i,j] = mean_b sigmoid(W2 . gelu(ctx[b] + cause[i] + effect[j] + b1) + b2)
with B=64, V=64, DIM=512, H=1024.

Sharding: data-parallel over B across 8 NeuronCores (8 batch rows per core);
embed / W1 / b1 / W2 / b2 are replicated. Each core emits the partial sum
sum_{b in shard} sigmoid(logits[b]) / 64 as a [1, 4096] tensor; the host sums
the 8 partials and reshapes to [64, 64].

Per-core plan (engines):
  - PE: phase-1 projections as N=512 float32r matmuls (cause_h/effect_h/ctx_h
        in natural layout, 1 cyc/row) + per-chunk transposes to the h-major
        layout; main-loop logits contraction vs W2 (bf16, N=512) with the 8
        slices spread over PE column groups via tile_position so consecutive
        matmuls overlap; tail ones-matmul sums the 8 local batch rows.
  - DVE: builds the pairwise table P[c][h, i, j] = cause[h,i] + effect[h,j]
        (broadcast tensor_tensor, bf16 out), PSUM->SBUF logits copies.
  - ACT (the roofline engine, ~240us/core): 64 x gelu(P[c] + CB[c,b]) over
        [128 x 4096], plus one final sigmoid over [8 x 4096].
"""

import sys

if "/opt/trn_rl_repo" not in sys.path:
    sys.path.insert(0, "/opt/trn_rl_repo")

import numpy as np

B, V, DIM = 64, 64, 512
H = 2 * DIM
N_CORES = 8
BS = B // N_CORES          # 8 batch rows per core
KC = DIM // 128            # 4 contraction chunks
HC = H // 128              # 8 hidden chunks
IJ = V * V                 # 4096

_CACHE = {}


def _build_nc():
    import concourse.bacc as bacc
    import concourse.bass as bass
    import concourse.mybir as mybir
    import concourse.tile as tile
    from concourse.masks import make_identity

    f32 = mybir.dt.float32
    f32r = mybir.dt.float32r
    bf16 = mybir.dt.bfloat16
    Gelu = mybir.ActivationFunctionType.Gelu
    Sigmoid = mybir.ActivationFunctionType.Sigmoid

    nc = bacc.Bacc("TRN2", target_bir_lowering=False, debug=False)

    st_d = nc.dram_tensor("state_s", [BS, DIM], f32, kind="ExternalInput")
    ac_d = nc.dram_tensor("action_s", [BS, DIM], f32, kind="ExternalInput")
    em_d = nc.dram_tensor("embed", [V, DIM], f32, kind="ExternalInput")
    w1_d = nc.dram_tensor("W1", [3 * DIM, H], f32, kind="ExternalInput")
    b1_d = nc.dram_tensor("b1", [H], f32, kind="ExternalInput")
    w2_d = nc.dram_tensor("W2", [H, 1], f32, kind="ExternalInput")
    b2_d = nc.dram_tensor("b2", [1], f32, kind="ExternalInput")
    out_d = nc.dram_tensor("out", [1, IJ], f32, kind="ExternalOutput")

    with tile.TileContext(nc) as tc:
        with (
            tc.tile_pool(name="singles", bufs=1) as singles,
            tc.tile_pool(name="wpool", bufs=2) as wpool,
            tc.tile_pool(name="actp", bufs=4) as actp,
            tc.tile_pool(name="scrp", bufs=2) as scrp,
        ):
            # W1 row-blocks first: 12 big contiguous DMAs on the gpsimd queue
            # so the weights stream in while the small loads/transposes run.
            wt = {}
            for mat in range(3):            # 0=cause(Wc) 1=effect(We) 2=ctx(Wx)
                for k in range(KC):
                    t = wpool.tile([128, H], f32r, tag=f"w{mat}", name=f"w{mat}_{k}")
                    nc.gpsimd.dma_start(
                        out=t[:, :],
                        in_=w1_d[mat * DIM + k * 128:mat * DIM + (k + 1) * 128, :],
                    )
                    wt[(mat, k)] = t

            ident = singles.tile([128, 128], f32)
            make_identity(nc, ident[:, :])

            e_raw = singles.tile([V, DIM], f32)
            nc.sync.dma_start(out=e_raw[:, :], in_=em_d[:, :])
            st_raw = singles.tile([BS, DIM], f32)
            nc.sync.dma_start(out=st_raw[:, :], in_=st_d[:, :])
            ac_raw = singles.tile([BS, DIM], f32)
            nc.sync.dma_start(out=ac_raw[:, :], in_=ac_d[:, :])
            # b1 / W2 loaded chunk-column-major: col c = elements [128c, 128c+128)
            b1_sb = singles.tile([128, HC], f32)
            nc.sync.dma_start(
                out=b1_sb[:, :], in_=b1_d.rearrange("(c p) -> p c", p=128)
            )
            w2_sb = singles.tile([128, HC], f32)
            nc.sync.dma_start(
                out=w2_sb[:, :], in_=w2_d.rearrange("(c p) o -> p (c o)", p=128)
            )
            b2_sb = singles.tile([BS, 1], f32)
            nc.sync.dma_start(out=b2_sb[:, :], in_=b2_d[:].to_broadcast((BS, 1)))

            w2_bf = singles.tile([128, HC], bf16)
            nc.vector.tensor_copy(out=w2_bf[:, :], in_=w2_sb[:, :])

            sa = singles.tile([BS, DIM], f32)
            nc.vector.tensor_add(out=sa[:, :], in0=st_raw[:, :], in1=ac_raw[:, :])

            embT = singles.tile([128, KC, V], f32r)    # embed^T, k-chunked (f32r for PE)
            saT = singles.tile([128, KC, BS], f32r)    # (state+action)^T, k-chunked
            cause_sb = singles.tile([V, H], f32)       # embed @ Wc
            eff_sb = singles.tile([V, H], f32)         # embed @ We
            ctx_sb = singles.tile([BS, H], f32)        # (state+action) @ Wx
            P = singles.tile([128, HC, V, V], bf16)    # cause (+) effect pairwise table
            CB = singles.tile([128, HC, BS], f32)      # ctx_hT + b1, per-(chunk, b) bias
            L = singles.tile([BS, IJ], f32)            # logits, batch-major
            S = singles.tile([BS, IJ], bf16)           # sigmoid(logits)
            ones = singles.tile([BS, 1], bf16)
            nc.vector.memset(ones[:, :], 1.0)
            out_sb = singles.tile([1, IJ], f32)

            with tc.tile_pool(name="psum1", bufs=1, space=bass.MemorySpace.PSUM) as psum1:
                # transposes of embed / (state+action) -> k-chunked lhsT layout
                for k in range(KC):
                    pt = psum1.tile([128, V], f32, tag="pt", bufs=2)
                    nc.tensor.transpose(
                        out=pt[:, :],
                        in_=e_raw[:, k * 128:(k + 1) * 128],
                        identity=ident[:V, :V],
                    )
                    nc.vector.tensor_copy(out=embT[:, k, :], in_=pt[:, :])
                for k in range(KC):
                    pt2 = psum1.tile([128, BS], f32, tag="pt", bufs=2)
                    nc.tensor.transpose(
                        out=pt2[:, :],
                        in_=sa[:, k * 128:(k + 1) * 128],
                        identity=ident[:BS, :BS],
                    )
                    nc.vector.tensor_copy(out=saT[:, k, :], in_=pt2[:, :])

                # cause_h/effect_h/ctx_h as N=512 float32r matmuls (1 cyc/row)
                for mat, (rows, lhs_full, dst) in enumerate([
                    (V, embT, cause_sb),
                    (V, embT, eff_sb),
                    (BS, saT, ctx_sb),
                ]):
                    pp = psum1.tile([rows, H], f32, tag=f"pp{mat}", name=f"pp{mat}")
                    for k in range(KC):
                        for half in range(2):
                            nc.tensor.matmul(
                                pp[:, half * 512:(half + 1) * 512],
                                lhsT=lhs_full[:, k, :rows],
                                rhs=wt[(mat, k)][:, half * 512:(half + 1) * 512],
                                start=(k == 0), stop=(k == KC - 1),
                            )
                    nc.vector.tensor_copy(out=dst[:, :], in_=pp[:, :])

            with tc.tile_pool(name="psum1b", bufs=1, space=bass.MemorySpace.PSUM) as psum1b:
                # per h-chunk: transpose to h-major, build P and CB
                for c in range(HC):
                    tpc = psum1b.tile([128, V], f32, tag="tpc", bufs=2)
                    nc.tensor.transpose(
                        out=tpc[:, :],
                        in_=cause_sb[:, c * 128:(c + 1) * 128],
                        identity=ident[:V, :V],
                    )
                    cau = wpool.tile([128, V], f32, tag="cau", name=f"cau{c}", bufs=2)
                    nc.vector.tensor_copy(out=cau[:, :], in_=tpc[:, :])

                    tpe = psum1b.tile([128, V], f32, tag="tpe", bufs=2)
                    nc.tensor.transpose(
                        out=tpe[:, :],
                        in_=eff_sb[:, c * 128:(c + 1) * 128],
                        identity=ident[:V, :V],
                    )
                    # P[c][p, i, j] = effect[p, j] + cause[p, i]
                    # (DVE may read at most one non-scalar operand from PSUM)
                    nc.vector.tensor_add(
                        out=P[:, c, :, :],
                        in0=tpe[:, None, :].to_broadcast((128, V, V)),
                        in1=cau[:, :, None].to_broadcast((128, V, V)),
                    )

                    tpx = psum1b.tile([128, BS], f32, tag="tpx", bufs=2)
                    nc.tensor.transpose(
                        out=tpx[:, :],
                        in_=ctx_sb[:, c * 128:(c + 1) * 128],
                        identity=ident[:BS, :BS],
                    )
                    nc.vector.tensor_scalar_add(
                        out=CB[:, c, :], in0=tpx[:, :], scalar1=b1_sb[:, c:c + 1]
                    )

            with tc.tile_pool(name="psum2", bufs=8, space=bass.MemorySpace.PSUM) as psum2:
                for b in range(BS):
                    # slice s lives on PE column group g = s % 4 (tile_position)
                    # and in PSUM/SBUF partition 32*g, so consecutive matmuls
                    # hit distinct column groups and overlap in the array.
                    pls = [
                        psum2.tile([128, 512], f32, tag="pl", name=f"pl{b}_{s}")
                        for s in range(8)
                    ]
                    for c in range(HC):
                        act = actp.tile([128, V, V], bf16, tag="act")
                        nc.scalar.activation(
                            out=act[:, :, :],
                            in_=P[:, c, :, :],
                            func=Gelu,
                            bias=CB[:, c, b:b + 1],
                            scale=1.0,
                        )
                        for s in range(8):
                            g = s % 4
                            nc.tensor.matmul(
                                pls[s][32 * g:32 * g + 1, :],
                                lhsT=w2_bf[:, c:c + 1],
                                rhs=act[:, 8 * s:8 * (s + 1), :],
                                start=(c == 0), stop=(c == HC - 1),
                                tile_position=(0, 32 * g),
                            )
                    scr = scrp.tile([97, 1024], f32, tag="scr")
                    for s in range(8):
                        g, q = s % 4, s // 4
                        nc.vector.tensor_copy(
                            out=scr[32 * g:32 * g + 1, 512 * q:512 * (q + 1)],
                            in_=pls[s][32 * g:32 * g + 1, :],
                        )
                    # engines can only address SBUF partitions {0,32,64,96}; DMA
                    # is exempt, so place row b of L with SBUF->SBUF DMAs
                    for s in range(8):
                        g, q = s % 4, s // 4
                        nc.sync.dma_start(
                            out=L[b:b + 1, 512 * s:512 * (s + 1)],
                            in_=scr[32 * g:32 * g + 1, 512 * q:512 * (q + 1)],
                        )

                nc.scalar.activation(
                    out=S[:, :], in_=L[:, :], func=Sigmoid, bias=b2_sb[:, :], scale=1.0
                )
                for s in range(8):
                    po = psum2.tile([128, 512], f32, tag="pl", name=f"po{s}")
                    nc.tensor.matmul(
                        po[0:1, :], lhsT=ones[:, :], rhs=S[:, 512 * s:512 * (s + 1)],
                        start=True, stop=True,
                    )
                    nc.vector.tensor_scalar_mul(
                        out=out_sb[:, 512 * s:512 * (s + 1)], in0=po[0:1, :],
                        scalar1=1.0 / B,
                    )
                nc.sync.dma_start(out=out_d[:, :], in_=out_sb[:, :])

    nc.compile()
    return nc


def _get_nc():
    if "nc" not in _CACHE:
        _CACHE["nc"] = _build_nc()
    return _CACHE["nc"]


def _make_in_maps(inputs):
    state = np.ascontiguousarray(np.asarray(inputs["state"], dtype=np.float32))
    action = np.ascontiguousarray(np.asarray(inputs["action"], dtype=np.float32))
    embed = np.ascontiguousarray(np.asarray(inputs["embed"], dtype=np.float32))
    W1 = np.ascontiguousarray(np.asarray(inputs["W1"], dtype=np.float32))
    b1 = np.ascontiguousarray(np.asarray(inputs["b1"], dtype=np.float32))
    W2 = np.ascontiguousarray(np.asarray(inputs["W2"], dtype=np.float32))
    b2 = np.ascontiguousarray(np.asarray(inputs["b2"], dtype=np.float32))
    in_maps = []
    for c in range(N_CORES):
        in_maps.append({
            "state_s": np.ascontiguousarray(state[c * BS:(c + 1) * BS]),
            "action_s": np.ascontiguousarray(action[c * BS:(c + 1) * BS]),
            "embed": embed,
            "W1": W1,
            "b1": b1,
            "W2": W2,
            "b2": b2,
        })
    return in_maps


def _ensure_ntff_hook():
    """This image's antenv lacks axon_hooks; synthesize it from the boot shim
    so run_bass_kernel_spmd(trace=True) can capture NTFF profiles."""
    import types

    try:
        from antenv.axon_hooks import get_axon_ntff_profile_hook  # noqa: F401
        return True
    except ImportError:
        pass
    try:
        if "/root/.axon_site" not in sys.path:
            sys.path.insert(0, "/root/.axon_site")
        from trn_agent_boot.trn_boot import _ntff_profile_via_ctypes

        hook = _ntff_profile_via_ctypes("/opt/axon/libaxon_pjrt.so")
    except Exception:
        hook = None
    if hook is None:
        return False
    import antenv

    mod = types.ModuleType("antenv.axon_hooks")
    mod._hook = hook
    mod.get_axon_ntff_profile_hook = lambda: mod._hook

    def set_axon_ntff_profile_hook(h):
        mod._hook = h

    mod.set_axon_ntff_profile_hook = set_axon_ntff_profile_hook
    sys.modules["antenv.axon_hooks"] = mod
    antenv.axon_hooks = mod
    return True


def run_sharded(inputs, trace=False, **kwargs):
    """Run the SPMD kernel on 8 cores; returns (scores [V,V] f32, BassKernelResults)."""
    from concourse.bass_utils import run_bass_kernel_spmd

    if trace:
        _ensure_ntff_hook()
    nc = _get_nc()
    in_maps = _make_in_maps(inputs)
    res = run_bass_kernel_spmd(
        nc, in_maps, core_ids=list(range(N_CORES)), trace=trace, **kwargs
    )
    parts = np.stack([res.results[c]["out"].reshape(V, V) for c in range(N_CORES)])
    scores = parts.astype(np.float64).sum(axis=0).astype(np.float32)
    return scores, res


def kernel(**inputs) -> np.ndarray:
    scores, _ = run_sharded(inputs, trace=False)
    return scores


if __name__ == "__main__":
    rng = np.random.default_rng(0)
    demo = {
        "state": rng.standard_normal((B, DIM), dtype=np.float32),
        "action": rng.standard_normal((B, DIM), dtype=np.float32),
        "embed": rng.standard_normal((V, DIM), dtype=np.float32),
        "W1": (rng.standard_normal((3 * DIM, H)) * 0.05).astype(np.float32),
        "b1": (rng.standard_normal((H,)) * 0.05).astype(np.float32),
        "W2": (rng.standard_normal((H, 1)) * 0.05).astype(np.float32),
        "b2": (rng.standard_normal((1,)) * 0.05).astype(np.float32),
    }
    out = kernel(**demo)
    print(out.shape, out.dtype, out[:2, :4])
